# revision 37
# baseline (speedup 1.0000x reference)
"""Autoformer-style EncoderLayer for Trainium2, data-parallel over batch
across 8 NeuronCores. v4: engine-rebalanced mixed-precision kernel.

  - decomp (banded matmul) + PE transposes in bf16; PE warmup matmuls
    during the input-DMA head (pstate ramp)
  - Q/K/V/WO projections, scores, attn@V, softmax denominator AND both
    FFN matmuls in fp8e4m3 DoubleRow (256-deep contraction)
  - K bias dropped entirely (softmax over m is invariant to the per-row
    constant Q'.bk)
  - LayerNorm statistics via DVE bn_stats/bn_aggr (single pass) instead
    of Scalar square+accumulate
  - exp on Scalar (native, direct fp8 out) with a few tiles on Vector
    via int16 Schraudolph fast-exp to balance the scores window
  - attention out bias bo' = bv@wo + bo applied exactly via a per-row
    den_l * bo'_d rank-1 matmul into the WO psum; bb2 via ones x bb2
    rank-1 matmul in FFN2
  - output DMA alternates between the SP and Pool queues

Per core: one [L=2048, D=512] sequence.
"""
import math
import numpy as np
import ml_dtypes
from contextlib import ExitStack

import concourse.bass as bass
import concourse.mybir as mybir
import concourse.tile as tile
from concourse import bacc
from concourse.bass_utils import run_bass_kernel_spmd

P = 128
B_, L, D = 8, 2048, 512
KPOOL, PAD = 25, 12
EPS = 1e-5
WS = 16.0                      # fp8 scale for wq/wk/wv/wo/w1/w2
SCALE8 = 1.0 / (math.sqrt(D) * WS * WS)
ESHIFT = -1.5                  # softmax shift: exp(s-1.5); avoids fp8 overflow
AVS = 1.0 / 256.0              # AV psum -> fp8 scale
HS = 16.0                      # h -> fp8 scale (FFN inputs)
FS = 1.0 / 256.0               # FFN2 psum -> residual scale (1/(HS*WS))
NLC = L // P          # 16 l-chunks of 128
NB = L // 512         # 4  l-blocks of 512
ND = D // P           # 4  d-chunks of 128

LN2C = math.log(2.0)
FXA = 128.0 / LN2C             # fast-exp: bf16bits = FXA*x + FXB
FXB = 127.0 * 128.0 - 7.0      # mantissa correction (Schraudolph, bf16)

f32 = mybir.dt.float32
bf16 = mybir.dt.bfloat16
f8 = mybir.dt.float8e4
i16 = mybir.dt.int16
AF = mybir.ActivationFunctionType
ALU = mybir.AluOpType
DR = mybir.MatmulPerfMode.DoubleRow

_CACHE = {}


def _band_blocks():
    i = np.arange(P)[:, None]
    j = np.arange(P)[None, :]
    a = (np.abs(i - j) <= PAD).astype(np.float32) / KPOOL
    bdiag = np.eye(P, dtype=np.float32) - a
    bup = -((i - j) >= (P - PAD)).astype(np.float32) / KPOOL
    bdown = bup.T.copy()
    return bdiag, bup, bdown


def _build(apply_g1, apply_g2):
    nc = bacc.Bacc("TRN2", target_bir_lowering=False, debug=False)

    x = nc.dram_tensor("x", [L, D], bf16, kind="ExternalInput").ap()
    w8 = {n: nc.dram_tensor(n, [P, ND, D], f8, kind="ExternalInput").ap()
          for n in ["wq8", "wk8", "wv8", "wo8"]}
    wb = {n: nc.dram_tensor(n, [P, ND, D], bf16, kind="ExternalInput").ap()
          for n in ["w1b", "w2b"]}
    ncb = 1664 + (1024 if apply_g1 else 0) + (1024 if apply_g2 else 0)
    cb16 = nc.dram_tensor("cb16", [P, ncb], bf16, kind="ExternalInput").ap()
    cf32 = nc.dram_tensor("cf32", [P, 16], f32, kind="ExternalInput").ap()

    out = nc.dram_tensor("out", [L, D], f32, kind="ExternalOutput").ap()
    out_c = out.rearrange("(l p) d -> l p d", p=P)

    with tile.TileContext(nc) as tc, ExitStack() as ctx:
        misc = ctx.enter_context(tc.tile_pool(name="misc", bufs=1))
        small = ctx.enter_context(tc.tile_pool(name="small", bufs=4))
        psum = ctx.enter_context(tc.tile_pool(name="psum", bufs=8, space="PSUM"))

        def mmtile(name):
            return psum.tile([P, 512], f32, tag="mm", name=name)

        def trtile(name):
            return psum.tile([P, ND, P], bf16, tag="mm", name=name)

        # ---- PE warmup: ramp the pstate while input DMAs land ----
        t_warm = misc.tile([P, 512], bf16, name="t_warm")
        nc.vector.memset(t_warm[:], 0.02)

        # ---- constants (bands first so decomp can start ASAP) ----
        t_cb = misc.tile([P, ncb], bf16, name="t_cb")
        nc.scalar.dma_start(t_cb[:, 0:512], cb16[:, 0:512])
        t_cf = misc.tile([P, 16], f32, name="t_cf")
        nc.scalar.dma_start(t_cf[:], cf32)
        nc.gpsimd.dma_start(t_cb[:, 512:ncb], cb16[:, 512:ncb])
        t_bd = t_cb[:, 0:128]
        t_bu = t_cb[:, 128:256]
        t_bn = t_cb[:, 256:384]
        t_id = t_cb[:, 384:512]
        t_bor = t_cb[0:1, 512:1024]   # bo' = bv@wo + bo, row [1, 512]
        t_bb2r = t_cb[0:1, 1024:1536]  # bb2*256 row [1, 512]
        t_or = t_cb[0:1, 1536:1664]   # ones row [1, 128]
        off = 1664
        t_gb = {}
        if apply_g1:
            t_gb["g1"] = t_cb[:, off:off + 512]
            t_gb["be1"] = t_cb[:, off + 512:off + 1024]
            off += 1024
        if apply_g2:
            t_gb["g2"] = t_cb[:, off:off + 512]
            t_gb["be2"] = t_cb[:, off + 512:off + 1024]
            off += 1024
        t_bqs = t_cf[:, 0:4]
        t_b1s = t_cf[:, 8:12]
        t_eps = t_cf[:, 12:13]
        t_esh = t_cf[:, 15:16]        # exp shift
        t_ones8 = misc.tile([P, 2, 128], f8, name="t_ones8")
        nc.vector.memset(t_ones8[:], 1.0)

        for _ in range(12):
            pwarm = mmtile("pwarm")
            nc.tensor.matmul(pwarm[:], t_warm[:, 0:128], t_warm[:],
                             start=True, stop=True, skip_group_check=True)

        # ---- stats / small persistents ----
        t_rec = misc.tile([P, NLC], f32, name="t_rec")
        t_den = misc.tile([1, L], f32, name="t_den")
        t_den8 = misc.tile([1, L], bf16, name="t_den8")

        # ---- persistent activations ----
        t_s = misc.tile([P, NLC, D], bf16, name="t_s")       # seasonal (pure)
        t_tr = misc.tile([P, NLC, D], bf16, name="t_tr")     # trend

        x_cview = x.rearrange("(l p) d -> p l d", p=P)

        def ln_stats(pool, t_in, t_mv_slot):
            """bn_stats+bn_aggr on [P,512] -> t_mv_slot [P,2] (mean,var)."""
            t6 = pool.tile([P, 6], f32, tag="bn6", name="t_bn6")
            nc.vector.bn_stats(t6[:], t_in)
            nc.vector.bn_aggr(t_mv_slot, t6[:])

        def ln_coefs(t_mv4, name):
            """From [P,4,2] (mean,var): returns (istd [P,4], nmi [P,4])."""
            t_sd = small.tile([P, 4], f32, tag="lbsd", name=f"sd{name}")
            nc.scalar.activation(t_sd[:], t_mv4[:, :, 1], AF.Sqrt, bias=t_eps[:])
            t_istd = small.tile([P, 4], f32, tag="lbi", name=f"istd{name}")
            nc.vector.reciprocal(t_istd[:], t_sd[:])
            t_nmi = small.tile([P, 4], f32, tag="lbnmi", name=f"nmi{name}")
            nc.vector.scalar_tensor_tensor(t_nmi[:], t_mv4[:, :, 0], -1.0,
                                           t_istd[:], op0=ALU.mult, op1=ALU.mult)
            return t_istd, t_nmi

        es_qkv = ExitStack()
        es_rest = ExitStack()
        try:
            uvp = es_rest.enter_context(tc.tile_pool(name="uvp", bufs=1))
            t_v = uvp.tile([P, NLC, D], f8, name="t_v")
            t_u = uvp.tile([P, NLC, L], f8, name="t_u")
            qkvp = es_qkv.enter_context(tc.tile_pool(name="qkvp", bufs=1))
            t_st = qkvp.tile([P, ND, L], f8, name="t_st")
            t_qt = qkvp.tile([P, ND, L], f8, name="t_qt")
            t_kt = qkvp.tile([P, ND, L], f8, name="t_kt")

            # ================= Phase 1: decomp + S^T =================
            with tc.tile_pool(name="xin", bufs=16) as xin, \
                 tc.tile_pool(name="sstage", bufs=3) as sstage:
                x_ch = {}

                def get_x(j):
                    if j not in x_ch:
                        t = xin.tile([P, D], bf16, tag="xw", name=f"xw{j}")
                        eng = nc.sync if j < 8 else nc.gpsimd
                        eng.dma_start(t[:], x_cview[:, j, :])
                        x_ch[j] = t
                    return x_ch[j]

                for j in range(6):
                    get_x(j)
                for j in range(8, 16):
                    get_x(j)

                # weights AFTER the x prefetch so x wins the queue race
                t_w8 = {}
                for n, eng in [("wq8", nc.scalar), ("wk8", nc.scalar),
                               ("wv8", nc.gpsimd), ("wo8", nc.gpsimd)]:
                    t_w8[n] = misc.tile([P, ND, D], f8, name=f"t_{n}")
                    eng.dma_start(t_w8[n][:], w8[n])
                t_wb = {}
                for n, eng in [("w1b", nc.gpsimd), ("w2b", nc.gpsimd)]:
                    t_wb[n] = misc.tile([P, ND, D], bf16, name=f"t_{n}")
                    eng.dma_start(t_wb[n][:], wb[n])

                for lc in range(NLC):
                    pss = mmtile("pss")
                    nbrs = [(lc - 1, t_bu), (lc, t_bd), (lc + 1, t_bn)]
                    nbrs = [(j, t) for j, t in nbrs if 0 <= j < NLC]
                    for i, (j, tb) in enumerate(nbrs):
                        nc.tensor.matmul(pss[:, 0:D], tb, get_x(j)[:],
                                         start=(i == 0), stop=(i == len(nbrs) - 1))
                    nc.scalar.copy(t_s[:, lc, :], pss[:, 0:D])
                    nc.gpsimd.tensor_tensor(t_tr[:, lc, :], get_x(lc)[:],
                                            t_s[:, lc, :], ALU.subtract)
                    ptr = trtile("ptr")
                    for dc in range(ND):
                        nc.tensor.transpose(ptr[:, dc, :],
                                            t_s[:, lc, bass.ts(dc, P)], t_id)
                    nc.vector.tensor_copy(t_st[:, :, bass.ts(lc, P)], ptr[:])

            # ================= Phase 2a: QT (fp8 DR) =================
            ei = 0
            for lb in range(NB):
                pq = {}
                for kp in range(2):
                    for dc in range(ND):
                        if kp == 0:
                            pq[dc] = mmtile(f"pq{dc}")
                        nc.tensor.matmul(
                            pq[dc][:],
                            t_w8["wq8"][:, 2 * kp:2 * kp + 2, bass.ts(dc, P)],
                            t_st[:, 2 * kp:2 * kp + 2, bass.ts(lb, 512)],
                            start=(kp == 0), stop=(kp == 1), perf_mode=DR)
                for dc in range(ND):
                    if ei % 2 == 0:
                        nc.scalar.activation(t_qt[:, dc, bass.ts(lb, 512)],
                                             pq[dc][:], AF.Identity,
                                             bias=t_bqs[:, dc:dc + 1])
                    else:
                        nc.vector.tensor_scalar(t_qt[:, dc, bass.ts(lb, 512)],
                                                pq[dc][:], t_bqs[:, dc:dc + 1],
                                                None, op0=ALU.add)
                    ei += 1

            # ========= Phase 2b/3: KT + scores + exp, per m-block =========
            # K bias dropped: softmax over m is invariant to Q'.bk per row.
            fxp = es_qkv.enter_context(tc.tile_pool(name="fxp", bufs=3))
            for lbk in range(NB):
                pk = {}
                for kp in range(2):
                    for dc in range(ND):
                        if kp == 0:
                            pk[dc] = mmtile(f"pk{dc}")
                        nc.tensor.matmul(
                            pk[dc][:],
                            t_w8["wk8"][:, 2 * kp:2 * kp + 2, bass.ts(dc, P)],
                            t_st[:, 2 * kp:2 * kp + 2, bass.ts(lbk, 512)],
                            start=(kp == 0), stop=(kp == 1), perf_mode=DR)
                for dc in range(ND):
                    nc.vector.tensor_copy(t_kt[:, dc, bass.ts(lbk, 512)],
                                          pk[dc][:])
                for mc in range(4 * lbk, 4 * lbk + 4):
                    for lbp in range(2):
                        psc = {}
                        for kp in range(2):
                            for lb in (2 * lbp, 2 * lbp + 1):
                                if kp == 0:
                                    psc[lb] = mmtile(f"psc{lb}")
                                nc.tensor.matmul(
                                    psc[lb][:],
                                    t_kt[:, 2 * kp:2 * kp + 2, bass.ts(mc, P)],
                                    t_qt[:, 2 * kp:2 * kp + 2, bass.ts(lb, 512)],
                                    start=(kp == 0), stop=(kp == 1), perf_mode=DR)
                        for lb in (2 * lbp, 2 * lbp + 1):
                            if lb == 3 and mc % 4 != 3:
                                t_i = fxp.tile([P, 512], i16, tag="fx", name="t_fx")
                                nc.vector.tensor_scalar(
                                    t_i[:], psc[lb][:], FXA * SCALE8,
                                    FXB + FXA * ESHIFT, op0=ALU.mult, op1=ALU.add)
                                nc.vector.tensor_copy(t_u[:, mc, bass.ts(lb, 512)],
                                                      t_i[:].bitcast(bf16))
                            else:
                                nc.scalar.activation(t_u[:, mc, bass.ts(lb, 512)],
                                                     psc[lb][:], AF.Exp,
                                                     scale=SCALE8, bias=t_esh)
                    # V projection for this m-chunk, in the exp shadow
                    pv = mmtile("pv")
                    for kp in range(2):
                        nc.tensor.matmul(
                            pv[:], t_st[:, 2 * kp:2 * kp + 2, bass.ts(mc, P)],
                            t_w8["wv8"][:, 2 * kp:2 * kp + 2, :],
                            start=(kp == 0), stop=(kp == 1), perf_mode=DR)
                    nc.vector.tensor_copy(t_v[:, mc, :], pv[:])

            # ================= Phase 4: dens + rec =================
            pden = {}
            for mcp in range(8):
                for lb in range(NB):
                    if mcp == 0:
                        pden[lb] = mmtile(f"pden{lb}")
                    nc.tensor.matmul(
                        pden[lb][0:1, :], t_ones8[:, :, 0:1],
                        t_u[:, 2 * mcp:2 * mcp + 2, bass.ts(lb, 512)],
                        start=(mcp == 0), stop=(mcp == 7), perf_mode=DR)
            for lb in range(NB):
                nc.scalar.copy(t_den[:, bass.ts(lb, 512)], pden[lb][0:1, :])
                nc.vector.tensor_copy(t_den8[:, bass.ts(lb, 512)],
                                      pden[lb][0:1, :])
            prc = mmtile("prc")
            for c in range(NLC):
                nc.tensor.matmul(prc[:, 2 * c:2 * c + 2],
                                 t_den8[:, bass.ts(c, P)], t_or[:, 0:2],
                                 start=True, stop=True)
            t_recs = small.tile([P, NLC], f32, tag="recs", name="t_recs")
            nc.vector.tensor_copy(t_recs[:], prc[:, 0:2 * NLC:2])
            nc.vector.reciprocal(t_rec[:], t_recs[:])

            es_qkv.close()  # free st/qt/kt (+fxp)

            # ========= Phase 5-8: per-block interleaved pipeline =========
            # Per lb: WO(lb) -> AV(lb+1) -> FFN2(lb-1) -> FFN1(lb), so every
            # epilogue chain resolves under the next block's matmuls. The
            # final FFN2(3) pipelines its LN2 per-chunk.
            avtp = es_rest.enter_context(tc.tile_pool(name="avtp", bufs=6))
            ffnp = es_rest.enter_context(tc.tile_pool(name="ffnp", bufs=1))
            t_h = ffnp.tile([P, NLC, D], bf16, name="t_h")
            t_ht = ffnp.tile([P, ND, L], bf16, name="t_ht")
            t_rt = ffnp.tile([P, ND, L], bf16, name="t_rt")
            t_mv1 = ffnp.tile([P, NLC, 2], f32, name="t_mv1")
            t_mv2 = ffnp.tile([P, NLC, 2], f32, name="t_mv2")

            avt_tiles = {}

            def av_pass(lb, half):
                t_a = avtp.tile([P, 2, 512], f8, tag="avt", name=f"avt{half}")
                pav = {}
                for mcp in range(8):
                    for di, dc in enumerate((2 * half, 2 * half + 1)):
                        if mcp == 0:
                            pav[di] = mmtile(f"pav{di}")
                        nc.tensor.matmul(
                            pav[di][:],
                            t_v[:, 2 * mcp:2 * mcp + 2, bass.ts(dc, P)],
                            t_u[:, 2 * mcp:2 * mcp + 2, bass.ts(lb, 512)],
                            start=(mcp == 0), stop=(mcp == 7), perf_mode=DR)
                for di in range(2):
                    nc.scalar.activation(t_a[:, di, :], pav[di][:], AF.Copy,
                                         scale=AVS)
                avt_tiles[(lb, half)] = t_a

            with tc.tile_pool(name="rsst", bufs=10) as rsst, \
                 tc.tile_pool(name="fst", bufs=6) as fst, \
                 tc.tile_pool(name="ost", bufs=4) as ost, \
                 tc.tile_pool(name="bnp", bufs=4) as bnp:

                def stage_a(lb):
                    rs_list = []
                    for c in range(4):
                        lc = lb * 4 + c
                        pwo = mmtile("pwo")
                        for kp in range(2):
                            nc.tensor.matmul(
                                pwo[:],
                                avt_tiles[(lb, kp)][:, :, bass.ts(c, P)],
                                t_w8["wo8"][:, 2 * kp:2 * kp + 2, :],
                                start=(kp == 0), stop=False, perf_mode=DR)
                        nc.tensor.matmul(pwo[:], t_den8[:, bass.ts(lc, P)],
                                         t_bor, start=False, stop=True)
                        t_rs = rsst.tile([P, D], bf16, tag="rs", name="t_rs")
                        nc.vector.scalar_tensor_tensor(
                            t_rs[:], pwo[:], t_rec[:, lc:lc + 1], t_s[:, lc, :],
                            op0=ALU.mult, op1=ALU.add)
                        ln_stats(bnp, t_rs[:], t_mv1[:, lc, :])
                        rs_list.append(t_rs)
                    return rs_list

                def ln1_apply(lb, rs_list):
                    t_istd4, t_nmi4 = ln_coefs(t_mv1[:, lb * 4:lb * 4 + 4, :],
                                               "l1")
                    for c in range(4):
                        lc = lb * 4 + c
                        if c % 2 == 0:
                            nc.vector.tensor_scalar(t_h[:, lc, :], rs_list[c][:],
                                                    t_istd4[:, c:c + 1],
                                                    t_nmi4[:, c:c + 1],
                                                    op0=ALU.mult, op1=ALU.add)
                        else:
                            nc.scalar.activation(t_h[:, lc, :], rs_list[c][:],
                                                 AF.Identity,
                                                 scale=t_istd4[:, c:c + 1],
                                                 bias=t_nmi4[:, c:c + 1])
                        if apply_g1:
                            nc.vector.tensor_tensor(t_h[:, lc, :], t_h[:, lc, :],
                                                    t_gb["g1"], ALU.mult)
                            nc.vector.tensor_tensor(t_h[:, lc, :], t_h[:, lc, :],
                                                    t_gb["be1"], ALU.add)
                        ptr = trtile("ptr2")
                        for dc in range(ND):
                            nc.tensor.transpose(ptr[:, dc, :],
                                                t_h[:, lc, bass.ts(dc, P)], t_id)
                        nc.vector.tensor_copy(t_ht[:, :, bass.ts(lc, P)], ptr[:])

                def ffn1(lb):
                    for dc in range(ND):
                        pf = mmtile("pf")
                        for k in range(ND):
                            nc.tensor.matmul(
                                pf[:], t_wb["w1b"][:, k, bass.ts(dc, P)],
                                t_ht[:, k, bass.ts(lb, 512)],
                                start=(k == 0), stop=(k == ND - 1))
                        nc.scalar.activation(t_rt[:, dc, bass.ts(lb, 512)],
                                             pf[:], AF.Relu,
                                             bias=t_b1s[:, dc:dc + 1])

                def ffn2_chunk(lc):
                    pf2 = mmtile("pf2")
                    for k in range(ND):
                        nc.tensor.matmul(pf2[:], t_rt[:, k, bass.ts(lc, P)],
                                         t_wb["w2b"][:, k, :],
                                         start=(k == 0), stop=False)
                    nc.tensor.matmul(pf2[:], t_or[:, 0:128], t_bb2r,
                                     start=False, stop=True)
                    t_res = fst.tile([P, D], bf16, tag="res", name="t_res")
                    nc.vector.scalar_tensor_tensor(
                        t_res[:], pf2[:], 1.0, t_h[:, lc, :],
                        op0=ALU.mult, op1=ALU.add)
                    ln_stats(bnp, t_res[:], t_mv2[:, lc, :])
                    return t_res

                def ln2_out(lc, t_res, t_istd, t_nmi, vap=False, vtr=False,
                            dsync=False):
                    t_h2 = fst.tile([P, D], bf16, tag="h2", name="t_h2")
                    if vap or lc % 2 == 0:
                        nc.vector.tensor_scalar(t_h2[:], t_res[:],
                                                t_istd, t_nmi,
                                                op0=ALU.mult, op1=ALU.add)
                    else:
                        nc.scalar.activation(t_h2[:], t_res[:], AF.Identity,
                                             scale=t_istd, bias=t_nmi)
                    if apply_g2:
                        nc.vector.tensor_tensor(t_h2[:], t_h2[:],
                                                t_gb["g2"], ALU.mult)
                        nc.vector.tensor_tensor(t_h2[:], t_h2[:],
                                                t_gb["be2"], ALU.add)
                    t_out = ost.tile([P, D], f32, tag="o", name="t_out")
                    oeng = nc.vector if vtr else nc.gpsimd
                    oeng.tensor_tensor(t_out[:], t_h2[:],
                                       t_tr[:, lc, :], ALU.add)
                    deng = nc.sync if (dsync or lc % 2 == 0) else nc.gpsimd
                    deng.dma_start(out_c[lc], t_out[:])

                def ffn2(lb):
                    res_list = [ffn2_chunk(lb * 4 + c) for c in range(4)]
                    t_istd4, t_nmi4 = ln_coefs(t_mv2[:, lb * 4:lb * 4 + 4, :],
                                               "l2")
                    for c in range(4):
                        ln2_out(lb * 4 + c, res_list[c],
                                t_istd4[:, c:c + 1], t_nmi4[:, c:c + 1])

                # AV passes run ahead of stage A: extra hoisting at lb=0
                # covers the LN1(0) chain with independent PE work.
                av_sched = {0: [(1, 0), (1, 1), (2, 0)],
                            1: [(2, 1), (3, 0)],
                            2: [(3, 1)],
                            3: []}
                av_pass(0, 0)
                av_pass(0, 1)
                for lb in range(NB):
                    rs_list = stage_a(lb)
                    for blk, half in av_sched[lb]:
                        av_pass(blk, half)
                    ln1_apply(lb, rs_list)
                    if lb >= 1:
                        ffn2(lb - 1)
                    ffn1(lb)
                # final block: per-chunk LN2 pipeline
                for c in range(4):
                    lc = 12 + c
                    t_res = ffn2_chunk(lc)
                    t_sd1 = small.tile([P, 1], f32, tag="fsd", name="t_fsd")
                    nc.scalar.activation(t_sd1[:], t_mv2[:, lc, 1:2], AF.Sqrt,
                                         bias=t_eps[:])
                    t_is1 = small.tile([P, 1], f32, tag="fis", name="t_fis")
                    nc.vector.reciprocal(t_is1[:], t_sd1[:])
                    t_nm1 = small.tile([P, 1], f32, tag="fnm", name="t_fnm")
                    nc.vector.scalar_tensor_tensor(
                        t_nm1[:], t_mv2[:, lc, 0:1], -1.0, t_is1[:],
                        op0=ALU.mult, op1=ALU.mult)
                    ln2_out(lc, t_res, t_is1[:], t_nm1[:],
                            vap=(c >= 2), vtr=(c % 2 == 1), dsync=(c >= 2))
        finally:
            es_qkv.close()
            es_rest.close()

    nc.compile()
    return nc


def _consts(inp, apply_g1, apply_g2):
    bdiag, bup, bdown = _band_blocks()
    ncb = 1664 + (1024 if apply_g1 else 0) + (1024 if apply_g2 else 0)
    cb = np.zeros((P, ncb), np.float32)
    cb[:, 0:128] = bdiag
    cb[:, 128:256] = bup
    cb[:, 256:384] = bdown
    cb[:, 384:512] = np.eye(P, dtype=np.float32)
    bo_p = inp["bv"] @ inp["wo"] + inp["bo"]
    cb[:, 512:1024] = bo_p.reshape(1, D)
    cb[:, 1024:1536] = inp["bb2"].reshape(1, D)
    cb[:, 1536:1664] = 1.0
    off = 1664
    if apply_g1:
        cb[:, off:off + 512] = inp["g1"].reshape(1, D)
        cb[:, off + 512:off + 1024] = inp["be1"].reshape(1, D)
        off += 1024
    if apply_g2:
        cb[:, off:off + 512] = inp["g2"].reshape(1, D)
        cb[:, off + 512:off + 1024] = inp["be2"].reshape(1, D)
        off += 1024
    cf = np.zeros((P, 16), np.float32)
    cf[:, 0:4] = (inp["bq"] * WS).reshape(ND, P).T
    cf[:, 8:12] = inp["bb1"].reshape(ND, P).T
    cf[:, 12] = EPS
    cf[:, 13:15] = 1.0
    cf[:, 15] = ESHIFT

    def pack_w(w):
        return np.ascontiguousarray(w.reshape(ND, P, D).transpose(1, 0, 2))

    consts = {
        "wq8": (pack_w(inp["wq"]) * WS).astype(ml_dtypes.float8_e4m3fn),
        "wk8": (pack_w(inp["wk"]) * WS).astype(ml_dtypes.float8_e4m3fn),
        "wv8": (pack_w(inp["wv"]) * WS).astype(ml_dtypes.float8_e4m3fn),
        "wo8": (pack_w(inp["wo"]) * WS).astype(ml_dtypes.float8_e4m3fn),
        "w1b": pack_w(inp["w1"]).astype(ml_dtypes.bfloat16),
        "w2b": pack_w(inp["w2"]).astype(ml_dtypes.bfloat16),
        "cb16": cb.astype(ml_dtypes.bfloat16),
        "cf32": cf,
    }
    return consts


def _prepare(inputs):
    inp = {k: np.ascontiguousarray(np.asarray(v, dtype=np.float32))
           for k, v in inputs.items()}
    x = inp["x"]                      # [8, 2048, 512]
    assert x.shape == (B_, L, D)

    apply_g1 = not (np.allclose(inp["g1"], 1.0) and np.allclose(inp["be1"], 0.0))
    apply_g2 = not (np.allclose(inp["g2"], 1.0) and np.allclose(inp["be2"], 0.0))

    key = (apply_g1, apply_g2)
    if key not in _CACHE:
        _CACHE[key] = _build(apply_g1, apply_g2)
    nc = _CACHE[key]

    consts = _consts(inp, apply_g1, apply_g2)
    x8 = x.astype(ml_dtypes.bfloat16)
    in_maps = [dict(consts, x=np.ascontiguousarray(x8[i])) for i in range(B_)]
    return nc, in_maps


def kernel(**inputs):
    nc, in_maps = _prepare(inputs)
    res = run_bass_kernel_spmd(nc, in_maps, core_ids=list(range(B_)))
    return np.stack([res.results[i]["out"] for i in range(B_)], axis=0)


# revision 40
# speedup vs baseline: 1.0868x; 1.0868x over previous
"""Autoformer-style EncoderLayer for Trainium2, data-parallel over batch
across 8 NeuronCores. v4: engine-rebalanced mixed-precision kernel.

  - decomp (banded matmul) + PE transposes in bf16; PE warmup matmuls
    during the input-DMA head (pstate ramp)
  - Q/K/V/WO projections, scores, attn@V, softmax denominator AND both
    FFN matmuls in fp8e4m3 DoubleRow (256-deep contraction)
  - K bias dropped entirely (softmax over m is invariant to the per-row
    constant Q'.bk)
  - LayerNorm statistics via DVE bn_stats/bn_aggr (single pass) instead
    of Scalar square+accumulate
  - exp on Scalar (native, direct fp8 out) with a few tiles on Vector
    via int16 Schraudolph fast-exp to balance the scores window
  - attention out bias bo' = bv@wo + bo applied exactly via a per-row
    den_l * bo'_d rank-1 matmul into the WO psum; bb2 via ones x bb2
    rank-1 matmul in FFN2
  - output DMA alternates between the SP and Pool queues

Per core: one [L=2048, D=512] sequence.
"""
import math
import numpy as np
import ml_dtypes
from contextlib import ExitStack

import concourse.bass as bass
import concourse.mybir as mybir
import concourse.tile as tile
from concourse import bacc
from concourse.bass_utils import run_bass_kernel_spmd

P = 128
B_, L, D = 8, 2048, 512
KPOOL, PAD = 25, 12
EPS = 1e-5
WS = 16.0                      # fp8 scale for wq/wk/wv/wo/w1/w2
SCALE8 = 1.0 / (math.sqrt(D) * WS * WS)
ESHIFT = -1.5                  # softmax shift: exp(s-1.5); avoids fp8 overflow
AVS = 1.0 / 256.0              # AV psum -> fp8 scale
HS = 16.0                      # h -> fp8 scale (FFN inputs)
FS = 1.0 / 256.0               # FFN2 psum -> residual scale (1/(HS*WS))
NLC = L // P          # 16 l-chunks of 128
NB = L // 512         # 4  l-blocks of 512
ND = D // P           # 4  d-chunks of 128

LN2C = math.log(2.0)
FXA = 128.0 / LN2C             # fast-exp: bf16bits = FXA*x + FXB
FXB = 127.0 * 128.0 - 7.0      # mantissa correction (Schraudolph, bf16)

f32 = mybir.dt.float32
bf16 = mybir.dt.bfloat16
f8 = mybir.dt.float8e4
i16 = mybir.dt.int16
AF = mybir.ActivationFunctionType
ALU = mybir.AluOpType
DR = mybir.MatmulPerfMode.DoubleRow

_CACHE = {}


def _band_blocks():
    i = np.arange(P)[:, None]
    j = np.arange(P)[None, :]
    a = (np.abs(i - j) <= PAD).astype(np.float32) / KPOOL
    bdiag = np.eye(P, dtype=np.float32) - a
    bup = -((i - j) >= (P - PAD)).astype(np.float32) / KPOOL
    bdown = bup.T.copy()
    return bdiag, bup, bdown


def _build(apply_g1, apply_g2):
    nc = bacc.Bacc("TRN2", target_bir_lowering=False, debug=False)

    x = nc.dram_tensor("x", [L, D], bf16, kind="ExternalInput").ap()
    w8 = {n: nc.dram_tensor(n, [P, ND, D], f8, kind="ExternalInput").ap()
          for n in ["wq8", "wk8", "wv8", "wo8"]}
    wb = {n: nc.dram_tensor(n, [P, ND, D], bf16, kind="ExternalInput").ap()
          for n in ["w1b", "w2b"]}
    ncb = 1664 + (1024 if apply_g1 else 0) + (1024 if apply_g2 else 0)
    cb16 = nc.dram_tensor("cb16", [P, ncb], bf16, kind="ExternalInput").ap()
    cf32 = nc.dram_tensor("cf32", [P, 16], f32, kind="ExternalInput").ap()

    out = nc.dram_tensor("out", [L, D], f32, kind="ExternalOutput").ap()
    out_c = out.rearrange("(l p) d -> l p d", p=P)

    with tile.TileContext(nc) as tc, ExitStack() as ctx:
        misc = ctx.enter_context(tc.tile_pool(name="misc", bufs=1))
        small = ctx.enter_context(tc.tile_pool(name="small", bufs=4))
        psum = ctx.enter_context(tc.tile_pool(name="psum", bufs=6, space="PSUM"))
        pstr = ctx.enter_context(tc.tile_pool(name="pstr", bufs=2, space="PSUM"))

        def mmtile(name):
            return psum.tile([P, 512], f32, tag="mm", name=name)

        def trtile(name):
            return pstr.tile([P, ND, P], bf16, tag="tr", name=name)

        # ---- PE warmup: ramp the pstate while input DMAs land ----
        t_warm = misc.tile([P, 512], bf16, name="t_warm")
        nc.vector.memset(t_warm[:], 0.02)

        # ---- constants (bands first so decomp can start ASAP) ----
        t_cb = misc.tile([P, ncb], bf16, name="t_cb")
        nc.scalar.dma_start(t_cb[:, 0:512], cb16[:, 0:512])
        t_cf = misc.tile([P, 16], f32, name="t_cf")
        nc.scalar.dma_start(t_cf[:], cf32)
        nc.gpsimd.dma_start(t_cb[:, 512:ncb], cb16[:, 512:ncb])
        t_bd = t_cb[:, 0:128]
        t_bu = t_cb[:, 128:256]
        t_bn = t_cb[:, 256:384]
        t_id = t_cb[:, 384:512]
        t_bor = t_cb[0:1, 512:1024]   # bo' = bv@wo + bo, row [1, 512]
        t_bb2r = t_cb[0:1, 1024:1536]  # bb2*256 row [1, 512]
        t_or = t_cb[0:1, 1536:1664]   # ones row [1, 128]
        off = 1664
        t_gb = {}
        if apply_g1:
            t_gb["g1"] = t_cb[:, off:off + 512]
            t_gb["be1"] = t_cb[:, off + 512:off + 1024]
            off += 1024
        if apply_g2:
            t_gb["g2"] = t_cb[:, off:off + 512]
            t_gb["be2"] = t_cb[:, off + 512:off + 1024]
            off += 1024
        t_bqs = t_cf[:, 0:4]
        t_b1s = t_cf[:, 8:12]
        t_eps = t_cf[:, 12:13]
        t_esh = t_cf[:, 15:16]        # exp shift
        t_ones8 = misc.tile([P, 2, 128], f8, name="t_ones8")
        nc.vector.memset(t_ones8[:], 1.0)

        for _ in range(12):
            pwarm = mmtile("pwarm")
            nc.tensor.matmul(pwarm[:], t_warm[:, 0:128], t_warm[:],
                             start=True, stop=True, skip_group_check=True)

        # ---- stats / small persistents ----
        t_rec = misc.tile([P, NLC], f32, name="t_rec")
        t_den = misc.tile([1, L], f32, name="t_den")
        t_den8 = misc.tile([1, L], bf16, name="t_den8")

        # ---- persistent activations ----
        t_s = misc.tile([P, NLC, D], bf16, name="t_s")       # seasonal (pure)
        t_tr = misc.tile([P, NLC, D], bf16, name="t_tr")     # trend

        x_cview = x.rearrange("(l p) d -> p l d", p=P)

        def ln_stats(pool, t_in, t_mv_slot):
            """bn_stats+bn_aggr on [P,512] -> t_mv_slot [P,2] (mean,var)."""
            t6 = pool.tile([P, 6], f32, tag="bn6", name="t_bn6")
            nc.vector.bn_stats(t6[:], t_in)
            nc.vector.bn_aggr(t_mv_slot, t6[:])

        def ln_coefs(t_mv4, name):
            """From [P,4,2] (mean,var): returns (istd [P,4], nmi [P,4])."""
            t_sd = small.tile([P, 4], f32, tag="lbsd", name=f"sd{name}")
            nc.scalar.activation(t_sd[:], t_mv4[:, :, 1], AF.Sqrt, bias=t_eps[:])
            t_istd = small.tile([P, 4], f32, tag="lbi", name=f"istd{name}")
            nc.vector.reciprocal(t_istd[:], t_sd[:])
            t_nmi = small.tile([P, 4], f32, tag="lbnmi", name=f"nmi{name}")
            nc.vector.scalar_tensor_tensor(t_nmi[:], t_mv4[:, :, 0], -1.0,
                                           t_istd[:], op0=ALU.mult, op1=ALU.mult)
            return t_istd, t_nmi

        es_qkv = ExitStack()
        es_rest = ExitStack()
        try:
            uvp = es_rest.enter_context(tc.tile_pool(name="uvp", bufs=1))
            t_v = uvp.tile([P, NLC, D], f8, name="t_v")
            t_u = uvp.tile([P, NLC, L], f8, name="t_u")
            qkvp = es_qkv.enter_context(tc.tile_pool(name="qkvp", bufs=1))
            t_st = qkvp.tile([P, ND, L], f8, name="t_st")
            t_qt = qkvp.tile([P, ND, L], f8, name="t_qt")
            t_kt = qkvp.tile([P, ND, L], f8, name="t_kt")

            # ================= Phase 1: decomp + S^T =================
            with tc.tile_pool(name="xin", bufs=16) as xin, \
                 tc.tile_pool(name="sstage", bufs=3) as sstage:
                x_ch = {}

                def get_x(j):
                    if j not in x_ch:
                        t = xin.tile([P, D], bf16, tag="xw", name=f"xw{j}")
                        eng = nc.sync if j < 8 else nc.gpsimd
                        eng.dma_start(t[:], x_cview[:, j, :])
                        x_ch[j] = t
                    return x_ch[j]

                for j in range(6):
                    get_x(j)
                for j in range(8, 16):
                    get_x(j)

                # attention weights AFTER the x prefetch so x wins the
                # queue race; the big FFN weights are issued much later
                # (they're needed only in stage B) to keep the early
                # window under the DMA bandwidth cap.
                t_w8 = {}
                for n, eng in [("wq8", nc.scalar), ("wk8", nc.scalar),
                               ("wv8", nc.gpsimd), ("wo8", nc.gpsimd)]:
                    t_w8[n] = misc.tile([P, ND, D], f8, name=f"t_{n}")
                    eng.dma_start(t_w8[n][:], w8[n])
                t_wb = {}
                for n in ["w1b", "w2b"]:
                    t_wb[n] = misc.tile([P, ND, D], bf16, name=f"t_{n}")

                for lc in range(NLC):
                    pss = mmtile("pss")
                    nbrs = [(lc - 1, t_bu), (lc, t_bd), (lc + 1, t_bn)]
                    nbrs = [(j, t) for j, t in nbrs if 0 <= j < NLC]
                    for i, (j, tb) in enumerate(nbrs):
                        nc.tensor.matmul(pss[:, 0:D], tb, get_x(j)[:],
                                         start=(i == 0), stop=(i == len(nbrs) - 1))
                    nc.scalar.copy(t_s[:, lc, :], pss[:, 0:D])
                    nc.gpsimd.tensor_tensor(t_tr[:, lc, :], get_x(lc)[:],
                                            t_s[:, lc, :], ALU.subtract)
                    ptr = trtile("ptr")
                    for dc in range(ND):
                        nc.tensor.transpose(ptr[:, dc, :],
                                            t_s[:, lc, bass.ts(dc, P)], t_id)
                    nc.vector.tensor_copy(t_st[:, :, bass.ts(lc, P)], ptr[:])

            # ================= Phase 2a: QT (fp8 DR) =================
            ei = 0
            for lb in range(NB):
                pq = {}
                for kp in range(2):
                    for dc in range(ND):
                        if kp == 0:
                            pq[dc] = mmtile(f"pq{dc}")
                        nc.tensor.matmul(
                            pq[dc][:],
                            t_w8["wq8"][:, 2 * kp:2 * kp + 2, bass.ts(dc, P)],
                            t_st[:, 2 * kp:2 * kp + 2, bass.ts(lb, 512)],
                            start=(kp == 0), stop=(kp == 1), perf_mode=DR)
                for dc in range(ND):
                    if ei % 2 == 0:
                        nc.scalar.activation(t_qt[:, dc, bass.ts(lb, 512)],
                                             pq[dc][:], AF.Identity,
                                             bias=t_bqs[:, dc:dc + 1])
                    else:
                        nc.vector.tensor_scalar(t_qt[:, dc, bass.ts(lb, 512)],
                                                pq[dc][:], t_bqs[:, dc:dc + 1],
                                                None, op0=ALU.add)
                    ei += 1

            nc.gpsimd.dma_start(t_wb["w1b"][:], wb["w1b"])

            # ========= Phase 2b/3: KT + scores + exp, per m-block =========
            # K bias dropped: softmax over m is invariant to Q'.bk per row.
            fxp = es_qkv.enter_context(tc.tile_pool(name="fxp", bufs=3))
            for lbk in range(NB):
                if lbk == 2:
                    nc.gpsimd.dma_start(t_wb["w2b"][:], wb["w2b"])
                pk = {}
                for kp in range(2):
                    for dc in range(ND):
                        if kp == 0:
                            pk[dc] = mmtile(f"pk{dc}")
                        nc.tensor.matmul(
                            pk[dc][:],
                            t_w8["wk8"][:, 2 * kp:2 * kp + 2, bass.ts(dc, P)],
                            t_st[:, 2 * kp:2 * kp + 2, bass.ts(lbk, 512)],
                            start=(kp == 0), stop=(kp == 1), perf_mode=DR)
                for dc in range(ND):
                    nc.vector.tensor_copy(t_kt[:, dc, bass.ts(lbk, 512)],
                                          pk[dc][:])
                for mc in range(4 * lbk, 4 * lbk + 4):
                    for lbp in range(2):
                        psc = {}
                        for kp in range(2):
                            for lb in (2 * lbp, 2 * lbp + 1):
                                if kp == 0:
                                    psc[lb] = mmtile(f"psc{lb}")
                                nc.tensor.matmul(
                                    psc[lb][:],
                                    t_kt[:, 2 * kp:2 * kp + 2, bass.ts(mc, P)],
                                    t_qt[:, 2 * kp:2 * kp + 2, bass.ts(lb, 512)],
                                    start=(kp == 0), stop=(kp == 1), perf_mode=DR)
                        for lb in (2 * lbp, 2 * lbp + 1):
                            if lb == 3 and mc % 4 != 3:
                                t_i = fxp.tile([P, 512], i16, tag="fx", name="t_fx")
                                nc.vector.tensor_scalar(
                                    t_i[:], psc[lb][:], FXA * SCALE8,
                                    FXB + FXA * ESHIFT, op0=ALU.mult, op1=ALU.add)
                                nc.vector.tensor_copy(t_u[:, mc, bass.ts(lb, 512)],
                                                      t_i[:].bitcast(bf16))
                            else:
                                nc.scalar.activation(t_u[:, mc, bass.ts(lb, 512)],
                                                     psc[lb][:], AF.Exp,
                                                     scale=SCALE8, bias=t_esh)
                    # V projection for this m-chunk, in the exp shadow
                    pv = mmtile("pv")
                    for kp in range(2):
                        nc.tensor.matmul(
                            pv[:], t_st[:, 2 * kp:2 * kp + 2, bass.ts(mc, P)],
                            t_w8["wv8"][:, 2 * kp:2 * kp + 2, :],
                            start=(kp == 0), stop=(kp == 1), perf_mode=DR)
                    nc.vector.tensor_copy(t_v[:, mc, :], pv[:])

            # ================= Phase 4: dens + rec =================
            pden = {}
            for mcp in range(8):
                for lb in range(NB):
                    if mcp == 0:
                        pden[lb] = mmtile(f"pden{lb}")
                    nc.tensor.matmul(
                        pden[lb][0:1, :], t_ones8[:, :, 0:1],
                        t_u[:, 2 * mcp:2 * mcp + 2, bass.ts(lb, 512)],
                        start=(mcp == 0), stop=(mcp == 7), perf_mode=DR)
            for lb in range(NB):
                nc.scalar.copy(t_den[:, bass.ts(lb, 512)], pden[lb][0:1, :])
                nc.vector.tensor_copy(t_den8[:, bass.ts(lb, 512)],
                                      pden[lb][0:1, :])
            prc = mmtile("prc")
            for c in range(NLC):
                nc.tensor.matmul(prc[:, 2 * c:2 * c + 2],
                                 t_den8[:, bass.ts(c, P)], t_or[:, 0:2],
                                 start=True, stop=True)
            t_recs = small.tile([P, NLC], f32, tag="recs", name="t_recs")
            nc.vector.tensor_copy(t_recs[:], prc[:, 0:2 * NLC:2])
            nc.vector.reciprocal(t_rec[:], t_recs[:])

            es_qkv.close()  # free st/qt/kt (+fxp)

            # ========= Phase 5-8: per-block interleaved pipeline =========
            # Per lb: WO(lb) -> AV(lb+1) -> FFN2(lb-1) -> FFN1(lb), so every
            # epilogue chain resolves under the next block's matmuls. The
            # final FFN2(3) pipelines its LN2 per-chunk.
            avtp = es_rest.enter_context(tc.tile_pool(name="avtp", bufs=6))
            ffnp = es_rest.enter_context(tc.tile_pool(name="ffnp", bufs=1))
            t_h = ffnp.tile([P, NLC, D], bf16, name="t_h")
            t_ht = ffnp.tile([P, ND, L], bf16, name="t_ht")
            t_rt = ffnp.tile([P, ND, L], bf16, name="t_rt")
            t_mv1 = ffnp.tile([P, NLC, 2], f32, name="t_mv1")
            t_mv2 = ffnp.tile([P, NLC, 2], f32, name="t_mv2")

            avt_tiles = {}

            def av_pass(lb, half):
                t_a = avtp.tile([P, 2, 512], f8, tag="avt", name=f"avt{half}")
                pav = {}
                for mcp in range(8):
                    for di, dc in enumerate((2 * half, 2 * half + 1)):
                        if mcp == 0:
                            pav[di] = mmtile(f"pav{di}")
                        nc.tensor.matmul(
                            pav[di][:],
                            t_v[:, 2 * mcp:2 * mcp + 2, bass.ts(dc, P)],
                            t_u[:, 2 * mcp:2 * mcp + 2, bass.ts(lb, 512)],
                            start=(mcp == 0), stop=(mcp == 7), perf_mode=DR)
                for di in range(2):
                    nc.scalar.activation(t_a[:, di, :], pav[di][:], AF.Copy,
                                         scale=AVS)
                avt_tiles[(lb, half)] = t_a

            with tc.tile_pool(name="rsst", bufs=10) as rsst, \
                 tc.tile_pool(name="fst", bufs=6) as fst, \
                 tc.tile_pool(name="ost", bufs=4) as ost, \
                 tc.tile_pool(name="bnp", bufs=4) as bnp:

                def stage_a(lb):
                    rs_list = []
                    for c in range(4):
                        lc = lb * 4 + c
                        pwo = mmtile("pwo")
                        for kp in range(2):
                            nc.tensor.matmul(
                                pwo[:],
                                avt_tiles[(lb, kp)][:, :, bass.ts(c, P)],
                                t_w8["wo8"][:, 2 * kp:2 * kp + 2, :],
                                start=(kp == 0), stop=False, perf_mode=DR)
                        nc.tensor.matmul(pwo[:], t_den8[:, bass.ts(lc, P)],
                                         t_bor, start=False, stop=True)
                        t_rs = rsst.tile([P, D], bf16, tag="rs", name="t_rs")
                        nc.vector.scalar_tensor_tensor(
                            t_rs[:], pwo[:], t_rec[:, lc:lc + 1], t_s[:, lc, :],
                            op0=ALU.mult, op1=ALU.add)
                        ln_stats(bnp, t_rs[:], t_mv1[:, lc, :])
                        rs_list.append(t_rs)
                    return rs_list

                def ln1_apply(lb, rs_list):
                    t_istd4, t_nmi4 = ln_coefs(t_mv1[:, lb * 4:lb * 4 + 4, :],
                                               "l1")
                    for c in range(4):
                        lc = lb * 4 + c
                        if c % 2 == 0:
                            nc.vector.tensor_scalar(t_h[:, lc, :], rs_list[c][:],
                                                    t_istd4[:, c:c + 1],
                                                    t_nmi4[:, c:c + 1],
                                                    op0=ALU.mult, op1=ALU.add)
                        else:
                            nc.scalar.activation(t_h[:, lc, :], rs_list[c][:],
                                                 AF.Identity,
                                                 scale=t_istd4[:, c:c + 1],
                                                 bias=t_nmi4[:, c:c + 1])
                        if apply_g1:
                            nc.vector.tensor_tensor(t_h[:, lc, :], t_h[:, lc, :],
                                                    t_gb["g1"], ALU.mult)
                            nc.vector.tensor_tensor(t_h[:, lc, :], t_h[:, lc, :],
                                                    t_gb["be1"], ALU.add)
                        ptr = trtile("ptr2")
                        for dc in range(ND):
                            nc.tensor.transpose(ptr[:, dc, :],
                                                t_h[:, lc, bass.ts(dc, P)], t_id)
                        nc.vector.tensor_copy(t_ht[:, :, bass.ts(lc, P)], ptr[:])

                def ffn1(lb):
                    for dc in range(ND):
                        pf = mmtile("pf")
                        for k in range(ND):
                            nc.tensor.matmul(
                                pf[:], t_wb["w1b"][:, k, bass.ts(dc, P)],
                                t_ht[:, k, bass.ts(lb, 512)],
                                start=(k == 0), stop=(k == ND - 1))
                        nc.scalar.activation(t_rt[:, dc, bass.ts(lb, 512)],
                                             pf[:], AF.Relu,
                                             bias=t_b1s[:, dc:dc + 1])

                def ffn2_chunk(lc):
                    pf2 = mmtile("pf2")
                    for k in range(ND):
                        nc.tensor.matmul(pf2[:], t_rt[:, k, bass.ts(lc, P)],
                                         t_wb["w2b"][:, k, :],
                                         start=(k == 0), stop=False)
                    nc.tensor.matmul(pf2[:], t_or[:, 0:128], t_bb2r,
                                     start=False, stop=True)
                    t_res = fst.tile([P, D], bf16, tag="res", name="t_res")
                    nc.vector.scalar_tensor_tensor(
                        t_res[:], pf2[:], 1.0, t_h[:, lc, :],
                        op0=ALU.mult, op1=ALU.add)
                    ln_stats(bnp, t_res[:], t_mv2[:, lc, :])
                    return t_res

                def ln2_out(lc, t_res, t_istd, t_nmi, vap=False, vtr=False,
                            dsync=False):
                    t_h2 = fst.tile([P, D], bf16, tag="h2", name="t_h2")
                    if vap or lc % 2 == 0:
                        nc.vector.tensor_scalar(t_h2[:], t_res[:],
                                                t_istd, t_nmi,
                                                op0=ALU.mult, op1=ALU.add)
                    else:
                        nc.scalar.activation(t_h2[:], t_res[:], AF.Identity,
                                             scale=t_istd, bias=t_nmi)
                    if apply_g2:
                        nc.vector.tensor_tensor(t_h2[:], t_h2[:],
                                                t_gb["g2"], ALU.mult)
                        nc.vector.tensor_tensor(t_h2[:], t_h2[:],
                                                t_gb["be2"], ALU.add)
                    t_out = ost.tile([P, D], f32, tag="o", name="t_out")
                    oeng = nc.vector if vtr else nc.gpsimd
                    oeng.tensor_tensor(t_out[:], t_h2[:],
                                       t_tr[:, lc, :], ALU.add)
                    deng = nc.sync if (dsync or lc % 2 == 0) else nc.gpsimd
                    deng.dma_start(out_c[lc], t_out[:])

                def ffn2(lb):
                    res_list = [ffn2_chunk(lb * 4 + c) for c in range(4)]
                    t_istd4, t_nmi4 = ln_coefs(t_mv2[:, lb * 4:lb * 4 + 4, :],
                                               "l2")
                    for c in range(4):
                        ln2_out(lb * 4 + c, res_list[c],
                                t_istd4[:, c:c + 1], t_nmi4[:, c:c + 1])

                # AV passes run ahead of stage A: extra hoisting at lb=0
                # covers the LN1(0) chain with independent PE work.
                av_sched = {0: [(1, 0), (1, 1), (2, 0)],
                            1: [(2, 1), (3, 0)],
                            2: [(3, 1)],
                            3: []}
                av_pass(0, 0)
                av_pass(0, 1)
                for lb in range(NB):
                    rs_list = stage_a(lb)
                    for blk, half in av_sched[lb]:
                        av_pass(blk, half)
                    ln1_apply(lb, rs_list)
                    if lb >= 1:
                        ffn2(lb - 1)
                    ffn1(lb)
                # final block: per-chunk LN2 pipeline
                for c in range(4):
                    lc = 12 + c
                    t_res = ffn2_chunk(lc)
                    t_sd1 = small.tile([P, 1], f32, tag="fsd", name="t_fsd")
                    nc.scalar.activation(t_sd1[:], t_mv2[:, lc, 1:2], AF.Sqrt,
                                         bias=t_eps[:])
                    t_is1 = small.tile([P, 1], f32, tag="fis", name="t_fis")
                    nc.vector.reciprocal(t_is1[:], t_sd1[:])
                    t_nm1 = small.tile([P, 1], f32, tag="fnm", name="t_fnm")
                    nc.vector.scalar_tensor_tensor(
                        t_nm1[:], t_mv2[:, lc, 0:1], -1.0, t_is1[:],
                        op0=ALU.mult, op1=ALU.mult)
                    ln2_out(lc, t_res, t_is1[:], t_nm1[:],
                            vap=(c >= 2), vtr=(c % 2 == 1), dsync=(c >= 2))
        finally:
            es_qkv.close()
            es_rest.close()

    nc.compile()
    return nc


def _consts(inp, apply_g1, apply_g2):
    bdiag, bup, bdown = _band_blocks()
    ncb = 1664 + (1024 if apply_g1 else 0) + (1024 if apply_g2 else 0)
    cb = np.zeros((P, ncb), np.float32)
    cb[:, 0:128] = bdiag
    cb[:, 128:256] = bup
    cb[:, 256:384] = bdown
    cb[:, 384:512] = np.eye(P, dtype=np.float32)
    bo_p = inp["bv"] @ inp["wo"] + inp["bo"]
    cb[:, 512:1024] = bo_p.reshape(1, D)
    cb[:, 1024:1536] = inp["bb2"].reshape(1, D)
    cb[:, 1536:1664] = 1.0
    off = 1664
    if apply_g1:
        cb[:, off:off + 512] = inp["g1"].reshape(1, D)
        cb[:, off + 512:off + 1024] = inp["be1"].reshape(1, D)
        off += 1024
    if apply_g2:
        cb[:, off:off + 512] = inp["g2"].reshape(1, D)
        cb[:, off + 512:off + 1024] = inp["be2"].reshape(1, D)
        off += 1024
    cf = np.zeros((P, 16), np.float32)
    cf[:, 0:4] = (inp["bq"] * WS).reshape(ND, P).T
    cf[:, 8:12] = inp["bb1"].reshape(ND, P).T
    cf[:, 12] = EPS
    cf[:, 13:15] = 1.0
    cf[:, 15] = ESHIFT

    def pack_w(w):
        return np.ascontiguousarray(w.reshape(ND, P, D).transpose(1, 0, 2))

    consts = {
        "wq8": (pack_w(inp["wq"]) * WS).astype(ml_dtypes.float8_e4m3fn),
        "wk8": (pack_w(inp["wk"]) * WS).astype(ml_dtypes.float8_e4m3fn),
        "wv8": (pack_w(inp["wv"]) * WS).astype(ml_dtypes.float8_e4m3fn),
        "wo8": (pack_w(inp["wo"]) * WS).astype(ml_dtypes.float8_e4m3fn),
        "w1b": pack_w(inp["w1"]).astype(ml_dtypes.bfloat16),
        "w2b": pack_w(inp["w2"]).astype(ml_dtypes.bfloat16),
        "cb16": cb.astype(ml_dtypes.bfloat16),
        "cf32": cf,
    }
    return consts


def _prepare(inputs):
    inp = {k: np.ascontiguousarray(np.asarray(v, dtype=np.float32))
           for k, v in inputs.items()}
    x = inp["x"]                      # [8, 2048, 512]
    assert x.shape == (B_, L, D)

    apply_g1 = not (np.allclose(inp["g1"], 1.0) and np.allclose(inp["be1"], 0.0))
    apply_g2 = not (np.allclose(inp["g2"], 1.0) and np.allclose(inp["be2"], 0.0))

    key = (apply_g1, apply_g2)
    if key not in _CACHE:
        _CACHE[key] = _build(apply_g1, apply_g2)
    nc = _CACHE[key]

    consts = _consts(inp, apply_g1, apply_g2)
    x8 = x.astype(ml_dtypes.bfloat16)
    in_maps = [dict(consts, x=np.ascontiguousarray(x8[i])) for i in range(B_)]
    return nc, in_maps


def kernel(**inputs):
    nc, in_maps = _prepare(inputs)
    res = run_bass_kernel_spmd(nc, in_maps, core_ids=list(range(B_)))
    return np.stack([res.results[i]["out"] for i in range(B_)], axis=0)
